# revision 8
# baseline (speedup 1.0000x reference)
"""Trainium2 Bass kernel for nn_GunnarODE: neural CDE with hermite spline control.

Contract: kernel(**inputs) takes FULL unsharded inputs (ts, us, ys, W1, b1,
W2, b2, batch_size) and returns the FULL (B, L, Y) output. Internally shards
the batch across 8 NeuronCores (pure data parallel), runs a Bass/Tile kernel
per core, and reassembles.

Algorithm notes (derived from the reference):
  - x = concat([t, us]) with unit-spaced knots (ts is arange) => dt == 1.
  - Hermite backward-difference spline derivative at substep s_i = i/4 of
    interval k reduces to dXdt_i = alpha_i * slope_{k-1} + beta_i * slope_k
    with alpha_i = 1-4s+3s^2, beta_i = 4s-3s^2 (alpha+beta=1), i.e. a linear
    blend of (u_{k-1}, u_k, u_{k+1}); the time channel has dXdt == 1.
  - Per Euler substep: h = tanh(z@W1.T+b1); vf = tanh(h@W2.T+b2) viewed as
    (Y=16, C=9); z += 0.25 * einsum(vf, dXdt).
  - On device everything is kept transposed (feature on partitions, batch on
    the free dim). The 144 vf rows are split into 128 "ctrl" rows
    (r=(c-1)*16+y for channels c=1..8) and 16 "time" rows (y*9).
  - All matmuls are fp32: the ODE amplifies per-step rounding ~1e5x, so
    reduced-precision matmuls (fp32r/bf16) fail the accuracy budget.
"""
import sys
if '/opt/trn_rl_repo' not in sys.path:
    sys.path.insert(0, '/opt/trn_rl_repo')

import numpy as np

N_CORES = 8
L = 512
B_TOT = 4096
U = 8
Y = 16
H = 128
C = U + 1
NI = L - 1            # intervals
HSTEP = 0.25          # dt / SUBSTEPS with dt == 1
B_LOC = B_TOT // N_CORES  # 512

ALPHA = [1.0, 0.1875, -0.25, -0.3125]
BETA = [0.0, 0.8125, 1.25, 1.3125]

_BUILD_CACHE = {}


def _host_constants(W1, b1, W2, b2):
    """Precompute transposed/permuted constant matrices (host-side, free)."""
    rowmap = np.array([(r % 16) * 9 + (r // 16 + 1) for r in range(128)])
    cst = {}
    cst["W1T"] = np.ascontiguousarray(W1.T)                        # (16,128)
    cst["W2aT"] = np.ascontiguousarray(W2[rowmap, :].T)            # (128,128)
    cst["W2bT"] = np.ascontiguousarray(W2[np.arange(16) * 9, :].T)  # (128,16)
    cst["b1c"] = np.ascontiguousarray(b1[:, None])                 # (128,1)
    cst["b2c"] = np.ascontiguousarray(b2[rowmap][:, None])         # (128,1)
    cst["b2t"] = np.ascontiguousarray(b2[np.arange(16) * 9][:, None])  # (16,1)
    # Abc32: one matmul computes the blended slopes for all 4 substeps:
    # dxall[i*8 + (c-1)] = -a_i*u_{k-1}[c] + (a_i-b_i)*u_k[c] + b_i*u_{k+1}[c]
    abc32 = np.zeros((24, 32), dtype=np.float32)
    for i in range(4):
        for cc in range(8):
            col = i * 8 + cc
            abc32[0 * 8 + cc, col] = -ALPHA[i]
            abc32[1 * 8 + cc, col] = ALPHA[i] - BETA[i]
            abc32[2 * 8 + cc, col] = BETA[i]
    cst["Abc32"] = abc32                                           # (24,32)
    sel = np.zeros((128, 16), dtype=np.float32)
    for r in range(128):
        sel[r, r % 16] = HSTEP
    cst["Sel"] = sel                                               # (128,16)
    return {k: v.astype(np.float32) for k, v in cst.items()}


def _build(n_intervals=NI):
    """Build + compile the Bass module (cached per interval count)."""
    key = n_intervals
    if key in _BUILD_CACHE:
        return _BUILD_CACHE[key]

    import concourse.bass as bass
    import concourse.bacc as bacc
    import concourse.tile as tile
    from concourse import mybir

    F32 = mybir.dt.float32
    TANH = mybir.ActivationFunctionType.Tanh
    MULT = mybir.AluOpType.mult
    ADD = mybir.AluOpType.add

    nc = bacc.Bacc("TRN2", target_bir_lowering=False, debug=False,
                   num_devices=N_CORES)

    d_us3 = nc.dram_tensor("us3", (n_intervals, 24, B_LOC), F32, kind="ExternalInput")
    d_ys0 = nc.dram_tensor("ys0T", (16, B_LOC), F32, kind="ExternalInput")
    d_W1T = nc.dram_tensor("W1T", (16, 128), F32, kind="ExternalInput")
    d_W2aT = nc.dram_tensor("W2aT", (128, 128), F32, kind="ExternalInput")
    d_W2bT = nc.dram_tensor("W2bT", (128, 16), F32, kind="ExternalInput")
    d_b1 = nc.dram_tensor("b1c", (128, 1), F32, kind="ExternalInput")
    d_b2c = nc.dram_tensor("b2c", (128, 1), F32, kind="ExternalInput")
    d_b2t = nc.dram_tensor("b2t", (16, 1), F32, kind="ExternalInput")
    d_Abc32 = nc.dram_tensor("Abc32", (24, 32), F32, kind="ExternalInput")
    d_Sel = nc.dram_tensor("Sel", (128, 16), F32, kind="ExternalInput")
    d_out = nc.dram_tensor("out", (n_intervals, 16, B_LOC), F32, kind="ExternalOutput")
    RB = 4  # dxall DRAM bounce ring slots
    d_bounce = nc.dram_tensor("dxbounce", (RB, 32, B_LOC), F32)

    with tile.TileContext(nc) as tc:
        with (
            tc.tile_pool(name="consts", bufs=1) as consts,
            tc.tile_pool(name="zpool", bufs=3) as zpool,
            tc.tile_pool(name="work", bufs=2) as work,
            tc.tile_pool(name="u3p", bufs=2) as u3p,
            tc.tile_pool(name="dxp", bufs=2) as dxp,
            tc.tile_pool(name="ps1", bufs=1, space="PSUM") as ps1,
            tc.tile_pool(name="ps2", bufs=2, space="PSUM") as ps2,
        ):
            W1T = consts.tile([16, 128], F32)
            W2aT = consts.tile([128, 128], F32)
            W2bT = consts.tile([128, 16], F32)
            b1c = consts.tile([128, 1], F32)
            b2c = consts.tile([128, 1], F32)
            b2t = consts.tile([16, 1], F32)
            Sel = consts.tile([128, 16], F32)
            Abc32 = consts.tile([24, 32], F32)
            nc.sync.dma_start(W1T[:], d_W1T.ap())
            nc.sync.dma_start(W2aT[:], d_W2aT.ap())
            nc.sync.dma_start(W2bT[:], d_W2bT.ap())
            nc.sync.dma_start(b1c[:], d_b1.ap())
            nc.sync.dma_start(b2c[:], d_b2c.ap())
            nc.sync.dma_start(b2t[:], d_b2t.ap())
            nc.sync.dma_start(Sel[:], d_Sel.ap())
            nc.sync.dma_start(Abc32[:], d_Abc32.ap())

            z = zpool.tile([16, B_LOC], F32, tag="z")
            nc.sync.dma_start(z[:], d_ys0.ap())

            def prepare_dx(k):
                """Blend slopes for interval k (all 4 substeps) and replicate
                each 8-channel group 16x along partitions via a DRAM bounce
                (DRAM access patterns support step-0 replication; SBUF ones
                don't). Runs one interval ahead of use."""
                u3 = u3p.tile([24, B_LOC], F32, tag="u3", name=f"u3_{k}")
                nc.sync.dma_start(u3[:], d_us3.ap()[k])
                dxall_ps = ps2.tile([32, B_LOC], F32, tag="dxall")
                nc.tensor.matmul(dxall_ps[:], Abc32[:], u3[:], start=True, stop=True)
                dxall_sb = work.tile([32, B_LOC], F32, tag="dxallsb")
                nc.vector.tensor_copy(dxall_sb[:], dxall_ps[:])
                slot = k % RB
                nc.sync.dma_start(d_bounce.ap()[slot], dxall_sb[:])
                dXball = dxp.tile([128, 4 * B_LOC], F32, tag="dxball", name=f"dxb_{k}")
                for i in range(4):
                    src = bass.AP(d_bounce, slot * 32 * B_LOC + i * 8 * B_LOC,
                                  [[B_LOC, 8], [0, 16], [1, B_LOC]])
                    nc.sync.dma_start(dXball[:, i * B_LOC:(i + 1) * B_LOC], src)
                return dXball

            # Two batch-split streams (columns halves): stream B's PE work
            # fills stream A's non-PE chain stages. fp32 matmul stream time is
            # bandwidth-proportional, so the split costs little extra PE.
            S = 2
            BS = B_LOC // S
            cs = [slice(s * BS, (s + 1) * BS) for s in range(S)]

            dx_q = [prepare_dx(0)]
            if n_intervals > 1:
                dx_q.append(prepare_dx(1))
            for k in range(n_intervals):
                dx_cur = dx_q.pop(0)
                for i in range(4):
                    hpre = ps1.tile([128, B_LOC], F32, tag="hpre")
                    th = work.tile([128, B_LOC], F32, tag="th")
                    vfc_ps = ps1.tile([128, B_LOC], F32, tag="vfc")
                    vft_ps = ps1.tile([16, B_LOC], F32, tag="vft")
                    vfc = work.tile([128, B_LOC], F32, tag="vfcs")
                    vft = work.tile([16, B_LOC], F32, tag="vfts")
                    tmp = work.tile([128, B_LOC], F32, tag="tmp")
                    zinc = ps2.tile([16, B_LOC], F32, tag="zinc")
                    t1 = work.tile([16, B_LOC], F32, tag="t1")
                    z_new = zpool.tile([16, B_LOC], F32, tag="z")
                    for s in range(S):
                        nc.tensor.matmul(hpre[:, cs[s]], W1T[:], z[:, cs[s]],
                                         start=True, stop=True)
                    for s in range(S):
                        nc.scalar.activation(th[:, cs[s]], hpre[:, cs[s]], TANH,
                                             bias=b1c[:])
                    for s in range(S):
                        nc.tensor.matmul(vfc_ps[:, cs[s]], W2aT[:], th[:, cs[s]],
                                         start=True, stop=True)
                        nc.tensor.matmul(vft_ps[:, cs[s]], W2bT[:], th[:, cs[s]],
                                         start=True, stop=True)
                    for s in range(S):
                        nc.scalar.activation(vfc[:, cs[s]], vfc_ps[:, cs[s]], TANH,
                                             bias=b2c[:])
                        nc.scalar.activation(vft[:, cs[s]], vft_ps[:, cs[s]], TANH,
                                             bias=b2t[:])
                    for s in range(S):
                        nc.vector.tensor_tensor(
                            tmp[:, cs[s]], vfc[:, cs[s]],
                            dx_cur[:, i * B_LOC + s * BS:i * B_LOC + (s + 1) * BS],
                            MULT)
                    for s in range(S):
                        nc.tensor.matmul(zinc[:, cs[s]], Sel[:], tmp[:, cs[s]],
                                         start=True, stop=True)
                    for s in range(S):
                        nc.vector.scalar_tensor_tensor(t1[:, cs[s]], vft[:, cs[s]],
                                                       HSTEP, zinc[:, cs[s]],
                                                       MULT, ADD)
                    for s in range(S):
                        nc.vector.tensor_tensor(z_new[:, cs[s]], t1[:, cs[s]],
                                                z[:, cs[s]], ADD)
                    z = z_new
                nc.sync.dma_start(d_out.ap()[k], z[:])
                if k + 2 < n_intervals:
                    dx_q.append(prepare_dx(k + 2))

    nc.compile()
    _BUILD_CACHE[key] = nc
    return nc


def _prep_core_inputs(us, ys, cst, core, n_intervals):
    b0 = core * B_LOC
    usc = np.ascontiguousarray(us[:, b0:b0 + B_LOC, :].transpose(0, 2, 1))  # (L,8,B)
    us_ext = np.concatenate([2.0 * usc[:1] - usc[1:2], usc], axis=0)  # (L+1,8,B)
    sw = np.lib.stride_tricks.sliding_window_view(us_ext, 3, axis=0)  # (L-1,8,B,3)
    us3 = np.ascontiguousarray(sw.transpose(0, 3, 1, 2).reshape(L - 1, 24, B_LOC))
    us3 = us3[:n_intervals].astype(np.float32)
    ys0T = np.ascontiguousarray(ys[0, b0:b0 + B_LOC, :].T).astype(np.float32)
    m = {"us3": us3, "ys0T": ys0T}
    m.update(cst)
    return m


def kernel(ts, us, ys, W1, b1, W2, b2, batch_size=None, n_intervals=NI):
    from concourse.bass_utils import run_bass_kernel_spmd

    us = np.asarray(us, dtype=np.float32)
    ys = np.asarray(ys, dtype=np.float32)
    cst = _host_constants(np.asarray(W1, np.float32), np.asarray(b1, np.float32),
                          np.asarray(W2, np.float32), np.asarray(b2, np.float32))
    nc = _build(n_intervals)
    in_maps = [_prep_core_inputs(us, ys, cst, c, n_intervals) for c in range(N_CORES)]
    res = run_bass_kernel_spmd(nc, in_maps, core_ids=list(range(N_CORES)))
    out = np.empty((B_TOT, n_intervals + 1, Y), dtype=np.float32)
    out[:, 0, :] = ys[0]
    for c in range(N_CORES):
        b0 = c * B_LOC
        out[b0:b0 + B_LOC, 1:, :] = res.results[c]["out"].transpose(2, 0, 1)
    kernel._last_results = res
    return out


# revision 11
# speedup vs baseline: 1.3834x; 1.3834x over previous
"""Trainium2 Bass kernel for nn_GunnarODE: neural CDE with hermite spline control.

Contract: kernel(**inputs) takes FULL unsharded inputs (ts, us, ys, W1, b1,
W2, b2, batch_size) and returns the FULL (B, L, Y) output. Internally shards
the batch across 8 NeuronCores (pure data parallel), runs a Bass/Tile kernel
per core, and reassembles.

Algorithm notes (derived from the reference):
  - x = concat([t, us]) with unit-spaced knots (ts is arange) => dt == 1.
  - Hermite backward-difference spline derivative at substep s_i = i/4 of
    interval k reduces to dXdt_i = alpha_i * slope_{k-1} + beta_i * slope_k
    with alpha_i = 1-4s+3s^2, beta_i = 4s-3s^2 (alpha+beta=1), i.e. a linear
    blend of (u_{k-1}, u_k, u_{k+1}); the time channel has dXdt == 1.
  - Per Euler substep: h = tanh(z@W1.T+b1); vf = tanh(h@W2.T+b2) viewed as
    (Y=16, C=9); z += 0.25 * einsum(vf, dXdt).
  - On device everything is kept transposed (feature on partitions, batch on
    the free dim). The 144 vf rows are split into 128 "ctrl" rows
    (r=(c-1)*16+y for channels c=1..8) and 16 "time" rows (y*9).
  - All matmuls are fp32: the ODE amplifies per-step rounding ~1e5x, so
    reduced-precision matmuls (fp32r/bf16) fail the accuracy budget.
"""
import sys
if '/opt/trn_rl_repo' not in sys.path:
    sys.path.insert(0, '/opt/trn_rl_repo')

import numpy as np

N_CORES = 8
L = 512
B_TOT = 4096
U = 8
Y = 16
H = 128
C = U + 1
NI = L - 1            # intervals
HSTEP = 0.25          # dt / SUBSTEPS with dt == 1
B_LOC = B_TOT // N_CORES  # 512

ALPHA = [1.0, 0.1875, -0.25, -0.3125]
BETA = [0.0, 0.8125, 1.25, 1.3125]

_BUILD_CACHE = {}


def _host_constants(W1, b1, W2, b2):
    """Precompute transposed/permuted constant matrices (host-side, free)."""
    rowmap = np.array([(r % 16) * 9 + (r // 16 + 1) for r in range(128)])
    cst = {}
    cst["W1T"] = np.ascontiguousarray(W1.T)                        # (16,128)
    cst["W2aT"] = np.ascontiguousarray(W2[rowmap, :].T)            # (128,128)
    cst["W2bT"] = np.ascontiguousarray(W2[np.arange(16) * 9, :].T)  # (128,16)
    cst["b1c"] = np.ascontiguousarray(b1[:, None])                 # (128,1)
    cst["b2c"] = np.ascontiguousarray(b2[rowmap][:, None])         # (128,1)
    cst["b2t"] = np.ascontiguousarray(b2[np.arange(16) * 9][:, None])  # (16,1)
    abc = np.zeros((4, 24, 128), dtype=np.float32)
    for i in range(4):
        for r in range(128):
            c = r // 16 + 1
            abc[i, 0 * 8 + c - 1, r] = -ALPHA[i]
            abc[i, 1 * 8 + c - 1, r] = ALPHA[i] - BETA[i]
            abc[i, 2 * 8 + c - 1, r] = BETA[i]
    cst["Abc"] = abc                                               # (4,24,128)
    sel = np.zeros((128, 16), dtype=np.float32)
    for r in range(128):
        sel[r, r % 16] = HSTEP
    cst["Sel"] = sel                                               # (128,16)
    return {k: v.astype(np.float32) for k, v in cst.items()}


def _build(n_intervals=NI):
    """Build + compile the Bass module (cached per interval count)."""
    key = n_intervals
    if key in _BUILD_CACHE:
        return _BUILD_CACHE[key]

    import concourse.bass as bass
    import concourse.bacc as bacc
    import concourse.tile as tile
    from concourse import mybir

    F32 = mybir.dt.float32
    TANH = mybir.ActivationFunctionType.Tanh
    MULT = mybir.AluOpType.mult
    ADD = mybir.AluOpType.add

    nc = bacc.Bacc("TRN2", target_bir_lowering=False, debug=False,
                   num_devices=N_CORES)

    d_us3 = nc.dram_tensor("us3", (n_intervals, 24, B_LOC), F32, kind="ExternalInput")
    d_ys0 = nc.dram_tensor("ys0T", (16, B_LOC), F32, kind="ExternalInput")
    d_W1T = nc.dram_tensor("W1T", (16, 128), F32, kind="ExternalInput")
    d_W2aT = nc.dram_tensor("W2aT", (128, 128), F32, kind="ExternalInput")
    d_W2bT = nc.dram_tensor("W2bT", (128, 16), F32, kind="ExternalInput")
    d_b1 = nc.dram_tensor("b1c", (128, 1), F32, kind="ExternalInput")
    d_b2c = nc.dram_tensor("b2c", (128, 1), F32, kind="ExternalInput")
    d_b2t = nc.dram_tensor("b2t", (16, 1), F32, kind="ExternalInput")
    d_Abc = nc.dram_tensor("Abc", (4, 24, 128), F32, kind="ExternalInput")
    d_Sel = nc.dram_tensor("Sel", (128, 16), F32, kind="ExternalInput")
    d_out = nc.dram_tensor("out", (n_intervals, 16, B_LOC), F32, kind="ExternalOutput")

    with tile.TileContext(nc) as tc:
        with (
            tc.tile_pool(name="consts", bufs=1) as consts,
            tc.tile_pool(name="zpool", bufs=3) as zpool,
            tc.tile_pool(name="work", bufs=2) as work,
            tc.tile_pool(name="u3p", bufs=3) as u3p,
            tc.tile_pool(name="ps1", bufs=1, space="PSUM") as ps1,
            tc.tile_pool(name="ps2", bufs=2, space="PSUM") as ps2,
        ):
            W1T = consts.tile([16, 128], F32)
            W2aT = consts.tile([128, 128], F32)
            W2bT = consts.tile([128, 16], F32)
            b1c = consts.tile([128, 1], F32)
            b2c = consts.tile([128, 1], F32)
            b2t = consts.tile([16, 1], F32)
            Sel = consts.tile([128, 16], F32)
            Abc = [consts.tile([24, 128], F32, name=f"Abc{i}") for i in range(4)]
            nc.sync.dma_start(W1T[:], d_W1T.ap())
            nc.sync.dma_start(W2aT[:], d_W2aT.ap())
            nc.sync.dma_start(W2bT[:], d_W2bT.ap())
            nc.sync.dma_start(b1c[:], d_b1.ap())
            nc.sync.dma_start(b2c[:], d_b2c.ap())
            nc.sync.dma_start(b2t[:], d_b2t.ap())
            nc.sync.dma_start(Sel[:], d_Sel.ap())
            for i in range(4):
                nc.sync.dma_start(Abc[i][:], d_Abc.ap()[i])

            z = zpool.tile([16, B_LOC], F32, tag="z")
            nc.sync.dma_start(z[:], d_ys0.ap())

            # hpre is a persistent PSUM accumulator holding W1 @ z (telescoped:
            # each substep adds W1 @ (z_{i+1} - z_i), so the final z-ADD and
            # the full K=16 re-projection drop off the critical chain).
            hpre = ps1.tile([128, B_LOC], F32, tag="hpre")
            nc.tensor.matmul(hpre[:], W1T[:], z[:], start=True, stop=False,
                             skip_group_check=True)

            for k in range(n_intervals):
                u3 = u3p.tile([24, B_LOC], F32, tag="u3")
                nc.sync.dma_start(u3[:], d_us3.ap()[k])
                for i in range(4):
                    dXb = ps2.tile([128, B_LOC], F32, tag="dXb")
                    nc.tensor.matmul(dXb[:], Abc[i][:], u3[:], start=True, stop=True)
                    th = work.tile([128, B_LOC], F32, tag="th")
                    nc.scalar.activation(th[:], hpre[:], TANH, bias=b1c[:])
                    vfc_ps = ps1.tile([128, B_LOC], F32, tag="vfc")
                    nc.tensor.matmul(vfc_ps[:], W2aT[:], th[:], start=True, stop=True)
                    vft_ps = ps1.tile([16, B_LOC], F32, tag="vft")
                    nc.tensor.matmul(vft_ps[:], W2bT[:], th[:], start=True, stop=True)
                    vfc = work.tile([128, B_LOC], F32, tag="vfcs")
                    nc.scalar.activation(vfc[:], vfc_ps[:], TANH, bias=b2c[:])
                    vft = work.tile([16, B_LOC], F32, tag="vfts")
                    nc.scalar.activation(vft[:], vft_ps[:], TANH, bias=b2t[:])
                    tmp = work.tile([128, B_LOC], F32, tag="tmp")
                    nc.vector.tensor_tensor(tmp[:], vfc[:], dXb[:], MULT)
                    zinc = ps2.tile([16, B_LOC], F32, tag="zinc")
                    nc.tensor.matmul(zinc[:], Sel[:], tmp[:], start=True, stop=True)
                    t1 = work.tile([16, B_LOC], F32, tag="t1")
                    nc.vector.scalar_tensor_tensor(t1[:], vft[:], HSTEP, zinc[:],
                                                   MULT, ADD)
                    # chain-critical: hpre += W1 @ (z_{i+1} - z_i)
                    nc.tensor.matmul(hpre[:], W1T[:], t1[:], start=False,
                                     stop=False, skip_group_check=True)
                    # off-chain bookkeeping: z_{i+1} = z_i + t1 (output path)
                    z_new = zpool.tile([16, B_LOC], F32, tag="z")
                    nc.vector.tensor_tensor(z_new[:], t1[:], z[:], ADD)
                    z = z_new
                nc.sync.dma_start(d_out.ap()[k], z[:])

    nc.compile()
    _BUILD_CACHE[key] = nc
    return nc


def _prep_core_inputs(us, ys, cst, core, n_intervals):
    b0 = core * B_LOC
    usc = np.ascontiguousarray(us[:, b0:b0 + B_LOC, :].transpose(0, 2, 1))  # (L,8,B)
    us_ext = np.concatenate([2.0 * usc[:1] - usc[1:2], usc], axis=0)  # (L+1,8,B)
    sw = np.lib.stride_tricks.sliding_window_view(us_ext, 3, axis=0)  # (L-1,8,B,3)
    us3 = np.ascontiguousarray(sw.transpose(0, 3, 1, 2).reshape(L - 1, 24, B_LOC))
    us3 = us3[:n_intervals].astype(np.float32)
    ys0T = np.ascontiguousarray(ys[0, b0:b0 + B_LOC, :].T).astype(np.float32)
    m = {"us3": us3, "ys0T": ys0T}
    m.update({k: v for k, v in cst.items() if k not in ("Abc",)})
    m["Abc"] = cst["Abc"]
    return m


def kernel(ts, us, ys, W1, b1, W2, b2, batch_size=None, n_intervals=NI):
    from concourse.bass_utils import run_bass_kernel_spmd

    us = np.asarray(us, dtype=np.float32)
    ys = np.asarray(ys, dtype=np.float32)
    cst = _host_constants(np.asarray(W1, np.float32), np.asarray(b1, np.float32),
                          np.asarray(W2, np.float32), np.asarray(b2, np.float32))
    nc = _build(n_intervals)
    in_maps = [_prep_core_inputs(us, ys, cst, c, n_intervals) for c in range(N_CORES)]
    res = run_bass_kernel_spmd(nc, in_maps, core_ids=list(range(N_CORES)))
    out = np.empty((B_TOT, n_intervals + 1, Y), dtype=np.float32)
    out[:, 0, :] = ys[0]
    for c in range(N_CORES):
        b0 = c * B_LOC
        out[b0:b0 + B_LOC, 1:, :] = res.results[c]["out"].transpose(2, 0, 1)
    kernel._last_results = res
    return out


# revision 14
# speedup vs baseline: 1.6535x; 1.1952x over previous
"""Trainium2 Bass kernel for nn_GunnarODE: neural CDE with hermite spline control.

Contract: kernel(**inputs) takes FULL unsharded inputs (ts, us, ys, W1, b1,
W2, b2, batch_size) and returns the FULL (B, L, Y) output. Internally shards
the batch across 8 NeuronCores (pure data parallel), runs a Bass/Tile kernel
per core, and reassembles.

Algorithm notes (derived from the reference):
  - x = concat([t, us]) with unit-spaced knots (ts is arange) => dt == 1.
  - Hermite backward-difference spline derivative at substep s_i = i/4 of
    interval k reduces to dXdt_i = alpha_i * slope_{k-1} + beta_i * slope_k
    with alpha_i = 1-4s+3s^2, beta_i = 4s-3s^2 (alpha+beta=1), i.e. a linear
    blend of (u_{k-1}, u_k, u_{k+1}); the time channel has dXdt == 1.
  - Per Euler substep: h = tanh(z@W1.T+b1); vf = tanh(h@W2.T+b2) viewed as
    (Y=16, C=9); z += 0.25 * einsum(vf, dXdt).
  - On device everything is kept transposed (feature on partitions, batch on
    the free dim). The 144 vf rows are split into 128 "ctrl" rows
    (r=(c-1)*16+y for channels c=1..8) and 16 "time" rows (y*9).
  - All matmuls are fp32: the ODE amplifies per-step rounding ~1e5x, so
    reduced-precision matmuls (fp32r/bf16) fail the accuracy budget.
"""
import sys
if '/opt/trn_rl_repo' not in sys.path:
    sys.path.insert(0, '/opt/trn_rl_repo')

import numpy as np

N_CORES = 8
L = 512
B_TOT = 4096
U = 8
Y = 16
H = 128
C = U + 1
NI = L - 1            # intervals
HSTEP = 0.25          # dt / SUBSTEPS with dt == 1
B_LOC = B_TOT // N_CORES  # 512

ALPHA = [1.0, 0.1875, -0.25, -0.3125]
BETA = [0.0, 0.8125, 1.25, 1.3125]

_BUILD_CACHE = {}


def _host_constants(W1, b1, W2, b2):
    """Precompute transposed/permuted constant matrices (host-side, free)."""
    rowmap = np.array([(r % 16) * 9 + (r // 16 + 1) for r in range(128)])
    cst = {}
    cst["W1T"] = np.ascontiguousarray(W1.T)                        # (16,128)
    cst["W2aT"] = np.ascontiguousarray(W2[rowmap, :].T)            # (128,128)
    cst["W2bT"] = np.ascontiguousarray(W2[np.arange(16) * 9, :].T)  # (128,16)
    cst["b1c"] = np.ascontiguousarray(b1[:, None])                 # (128,1)
    cst["b2c"] = np.ascontiguousarray(b2[rowmap][:, None])         # (128,1)
    cst["b2t"] = np.ascontiguousarray(b2[np.arange(16) * 9][:, None])  # (16,1)
    abc = np.zeros((4, 24, 128), dtype=np.float32)
    for i in range(4):
        for r in range(128):
            c = r // 16 + 1
            abc[i, 0 * 8 + c - 1, r] = -ALPHA[i]
            abc[i, 1 * 8 + c - 1, r] = ALPHA[i] - BETA[i]
            abc[i, 2 * 8 + c - 1, r] = BETA[i]
    cst["Abc"] = abc                                               # (4,24,128)
    # hpre-state update matrices: hpre += (h*W1*Sel^T) @ tmp + (h*W1) @ vft
    w1selt = np.zeros((128, 128), dtype=np.float32)  # [r, j] = h*W1[j, r%16]
    for r in range(128):
        w1selt[r, :] = HSTEP * W1[:, r % 16]
    cst["W1SelT"] = w1selt
    cst["W1hT"] = (HSTEP * W1.T)                                   # (16,128)
    # output reconstruction: z = pinv(W1) @ hpre  (W1 is 128x16, cond ~2)
    R = np.linalg.pinv(W1.astype(np.float64)).astype(np.float32)   # (16,128)
    cst["RT"] = np.ascontiguousarray(R.T)                          # (128,16)
    return {k: v.astype(np.float32) for k, v in cst.items()}


def _build(n_intervals=NI):
    """Build + compile the Bass module (cached per interval count)."""
    key = n_intervals
    if key in _BUILD_CACHE:
        return _BUILD_CACHE[key]

    import concourse.bass as bass
    import concourse.bacc as bacc
    import concourse.tile as tile
    from concourse import mybir

    F32 = mybir.dt.float32
    TANH = mybir.ActivationFunctionType.Tanh
    MULT = mybir.AluOpType.mult
    ADD = mybir.AluOpType.add

    nc = bacc.Bacc("TRN2", target_bir_lowering=False, debug=False,
                   num_devices=N_CORES)

    d_us3 = nc.dram_tensor("us3", (n_intervals, 24, B_LOC), F32, kind="ExternalInput")
    d_ys0 = nc.dram_tensor("ys0T", (16, B_LOC), F32, kind="ExternalInput")
    d_W1T = nc.dram_tensor("W1T", (16, 128), F32, kind="ExternalInput")
    d_W2aT = nc.dram_tensor("W2aT", (128, 128), F32, kind="ExternalInput")
    d_W2bT = nc.dram_tensor("W2bT", (128, 16), F32, kind="ExternalInput")
    d_b1 = nc.dram_tensor("b1c", (128, 1), F32, kind="ExternalInput")
    d_b2c = nc.dram_tensor("b2c", (128, 1), F32, kind="ExternalInput")
    d_b2t = nc.dram_tensor("b2t", (16, 1), F32, kind="ExternalInput")
    d_Abc = nc.dram_tensor("Abc", (4, 24, 128), F32, kind="ExternalInput")
    d_W1SelT = nc.dram_tensor("W1SelT", (128, 128), F32, kind="ExternalInput")
    d_W1hT = nc.dram_tensor("W1hT", (16, 128), F32, kind="ExternalInput")
    d_RT = nc.dram_tensor("RT", (128, 16), F32, kind="ExternalInput")
    d_out = nc.dram_tensor("out", (n_intervals, 16, B_LOC), F32, kind="ExternalOutput")

    with tile.TileContext(nc) as tc:
        with (
            tc.tile_pool(name="consts", bufs=1) as consts,
            tc.tile_pool(name="zpool", bufs=3) as zpool,
            tc.tile_pool(name="work", bufs=2) as work,
            tc.tile_pool(name="u3p", bufs=3) as u3p,
            tc.tile_pool(name="ps1", bufs=1, space="PSUM") as ps1,
            tc.tile_pool(name="ps2", bufs=2, space="PSUM") as ps2,
        ):
            W1T = consts.tile([16, 128], F32)
            W2aT = consts.tile([128, 128], F32)
            W2bT = consts.tile([128, 16], F32)
            b1c = consts.tile([128, 1], F32)
            b2c = consts.tile([128, 1], F32)
            b2t = consts.tile([16, 1], F32)
            W1SelT = consts.tile([128, 128], F32)
            W1hT = consts.tile([16, 128], F32)
            RT = consts.tile([128, 16], F32)
            Abc = [consts.tile([24, 128], F32, name=f"Abc{i}") for i in range(4)]
            nc.sync.dma_start(W1T[:], d_W1T.ap())
            nc.sync.dma_start(W2aT[:], d_W2aT.ap())
            nc.sync.dma_start(W2bT[:], d_W2bT.ap())
            nc.sync.dma_start(b1c[:], d_b1.ap())
            nc.sync.dma_start(b2c[:], d_b2c.ap())
            nc.sync.dma_start(b2t[:], d_b2t.ap())
            nc.sync.dma_start(W1SelT[:], d_W1SelT.ap())
            nc.sync.dma_start(W1hT[:], d_W1hT.ap())
            nc.sync.dma_start(RT[:], d_RT.ap())
            for i in range(4):
                nc.sync.dma_start(Abc[i][:], d_Abc.ap()[i])

            z0 = zpool.tile([16, B_LOC], F32, tag="z")
            nc.sync.dma_start(z0[:], d_ys0.ap())

            # hpre is THE state: a persistent PSUM accumulator holding W1 @ z.
            # Each substep adds W1 @ dz via one K=128 + one K=16 matmul; z is
            # only reconstructed per interval for output via R = pinv(W1).
            hpre = ps1.tile([128, B_LOC], F32, tag="hpre")
            nc.tensor.matmul(hpre[:], W1T[:], z0[:], start=True, stop=False,
                             skip_group_check=True)

            for k in range(n_intervals):
                u3 = u3p.tile([24, B_LOC], F32, tag="u3")
                nc.sync.dma_start(u3[:], d_us3.ap()[k])
                for i in range(4):
                    dXb = ps2.tile([128, B_LOC], F32, tag="dXb")
                    # dXb halves fill the tanh windows on the PE
                    nc.tensor.matmul(dXb[:, :B_LOC // 2], Abc[i][:],
                                     u3[:, :B_LOC // 2], start=True, stop=True)
                    th = work.tile([128, B_LOC], F32, tag="th")
                    nc.scalar.activation(th[:], hpre[:], TANH, bias=b1c[:])
                    vfc_ps = ps1.tile([128, B_LOC], F32, tag="vfc")
                    nc.tensor.matmul(vfc_ps[:], W2aT[:], th[:], start=True, stop=True)
                    vft_ps = ps1.tile([16, B_LOC], F32, tag="vft")
                    nc.tensor.matmul(vft_ps[:], W2bT[:], th[:], start=True, stop=True)
                    nc.tensor.matmul(dXb[:, B_LOC // 2:], Abc[i][:],
                                     u3[:, B_LOC // 2:], start=True, stop=True)
                    vfc = work.tile([128, B_LOC], F32, tag="vfcs")
                    nc.scalar.activation(vfc[:], vfc_ps[:], TANH, bias=b2c[:])
                    vft = work.tile([16, B_LOC], F32, tag="vfts")
                    nc.scalar.activation(vft[:], vft_ps[:], TANH, bias=b2t[:])
                    tmp = work.tile([128, B_LOC], F32, tag="tmp")
                    nc.vector.tensor_tensor(tmp[:], vfc[:], dXb[:], MULT)
                    # chain-critical state update:
                    # hpre += (h*W1*Sel^T)@tmp + (h*W1)@vft
                    nc.tensor.matmul(hpre[:], W1SelT[:], tmp[:], start=False,
                                     stop=False, skip_group_check=True)
                    nc.tensor.matmul(hpre[:], W1hT[:], vft[:], start=False,
                                     stop=False, skip_group_check=True)
                # per-interval output: z_{k+1} = pinv(W1) @ hpre
                hps = work.tile([128, B_LOC], F32, tag="hps")
                nc.vector.tensor_copy(hps[:], hpre[:])
                zt_ps = ps2.tile([16, B_LOC], F32, tag="ztp")
                nc.tensor.matmul(zt_ps[:], RT[:], hps[:], start=True, stop=True)
                zout = zpool.tile([16, B_LOC], F32, tag="z")
                nc.vector.tensor_copy(zout[:], zt_ps[:])
                nc.sync.dma_start(d_out.ap()[k], zout[:])

    nc.compile()
    _BUILD_CACHE[key] = nc
    return nc


def _prep_core_inputs(us, ys, cst, core, n_intervals):
    b0 = core * B_LOC
    usc = np.ascontiguousarray(us[:, b0:b0 + B_LOC, :].transpose(0, 2, 1))  # (L,8,B)
    us_ext = np.concatenate([2.0 * usc[:1] - usc[1:2], usc], axis=0)  # (L+1,8,B)
    sw = np.lib.stride_tricks.sliding_window_view(us_ext, 3, axis=0)  # (L-1,8,B,3)
    us3 = np.ascontiguousarray(sw.transpose(0, 3, 1, 2).reshape(L - 1, 24, B_LOC))
    us3 = us3[:n_intervals].astype(np.float32)
    ys0T = np.ascontiguousarray(ys[0, b0:b0 + B_LOC, :].T).astype(np.float32)
    m = {"us3": us3, "ys0T": ys0T}
    m.update({k: v for k, v in cst.items() if k not in ("Abc",)})
    m["Abc"] = cst["Abc"]
    return m


def kernel(ts, us, ys, W1, b1, W2, b2, batch_size=None, n_intervals=NI):
    from concourse.bass_utils import run_bass_kernel_spmd

    us = np.asarray(us, dtype=np.float32)
    ys = np.asarray(ys, dtype=np.float32)
    cst = _host_constants(np.asarray(W1, np.float32), np.asarray(b1, np.float32),
                          np.asarray(W2, np.float32), np.asarray(b2, np.float32))
    nc = _build(n_intervals)
    in_maps = [_prep_core_inputs(us, ys, cst, c, n_intervals) for c in range(N_CORES)]
    res = run_bass_kernel_spmd(nc, in_maps, core_ids=list(range(N_CORES)))
    out = np.empty((B_TOT, n_intervals + 1, Y), dtype=np.float32)
    out[:, 0, :] = ys[0]
    for c in range(N_CORES):
        b0 = c * B_LOC
        out[b0:b0 + B_LOC, 1:, :] = res.results[c]["out"].transpose(2, 0, 1)
    kernel._last_results = res
    return out


# revision 15
# speedup vs baseline: 1.9512x; 1.1800x over previous
"""Trainium2 Bass kernel for nn_GunnarODE: neural CDE with hermite spline control.

Contract: kernel(**inputs) takes FULL unsharded inputs (ts, us, ys, W1, b1,
W2, b2, batch_size) and returns the FULL (B, L, Y) output. Internally shards
the batch across 8 NeuronCores (pure data parallel), runs a Bass/Tile kernel
per core, and reassembles.

Algorithm notes (derived from the reference):
  - x = concat([t, us]) with unit-spaced knots (ts is arange) => dt == 1.
  - Hermite backward-difference spline derivative at substep s_i = i/4 of
    interval k reduces to dXdt_i = alpha_i * slope_{k-1} + beta_i * slope_k
    with alpha_i = 1-4s+3s^2, beta_i = 4s-3s^2 (alpha+beta=1), i.e. a linear
    blend of (u_{k-1}, u_k, u_{k+1}); the time channel has dXdt == 1.
  - Per Euler substep: h = tanh(z@W1.T+b1); vf = tanh(h@W2.T+b2) viewed as
    (Y=16, C=9); z += 0.25 * einsum(vf, dXdt).
  - On device everything is kept transposed (feature on partitions, batch on
    the free dim). The 144 vf rows are split into 128 "ctrl" rows
    (r=(c-1)*16+y for channels c=1..8) and 16 "time" rows (y*9).
  - All matmuls are fp32: the ODE amplifies per-step rounding ~1e5x, so
    reduced-precision matmuls (fp32r/bf16) fail the accuracy budget.
"""
import sys
if '/opt/trn_rl_repo' not in sys.path:
    sys.path.insert(0, '/opt/trn_rl_repo')

import numpy as np

N_CORES = 8
L = 512
B_TOT = 4096
U = 8
Y = 16
H = 128
C = U + 1
NI = L - 1            # intervals
HSTEP = 0.25          # dt / SUBSTEPS with dt == 1
B_LOC = B_TOT // N_CORES  # 512

ALPHA = [1.0, 0.1875, -0.25, -0.3125]
BETA = [0.0, 0.8125, 1.25, 1.3125]

_BUILD_CACHE = {}


def _host_constants(W1, b1, W2, b2):
    """Precompute transposed/permuted constant matrices (host-side, free)."""
    rowmap = np.array([(r % 16) * 9 + (r // 16 + 1) for r in range(128)])
    cst = {}
    cst["W1T"] = np.ascontiguousarray(W1.T)                        # (16,128)
    cst["W2aT"] = np.ascontiguousarray(W2[rowmap, :].T)            # (128,128)
    cst["W2bT"] = np.ascontiguousarray(W2[np.arange(16) * 9, :].T)  # (128,16)
    cst["b1c"] = np.ascontiguousarray(b1[:, None])                 # (128,1)
    cst["b2c"] = np.ascontiguousarray(b2[rowmap][:, None])         # (128,1)
    cst["b2t"] = np.ascontiguousarray(b2[np.arange(16) * 9][:, None])  # (16,1)
    abc = np.zeros((4, 24, 128), dtype=np.float32)
    for i in range(4):
        for r in range(128):
            c = r // 16 + 1
            abc[i, 0 * 8 + c - 1, r] = -ALPHA[i]
            abc[i, 1 * 8 + c - 1, r] = ALPHA[i] - BETA[i]
            abc[i, 2 * 8 + c - 1, r] = BETA[i]
    cst["Abc"] = abc                                               # (4,24,128)
    # hpre-state update matrices: hpre += (h*W1*Sel^T) @ tmp + (h*W1) @ vft
    w1selt = np.zeros((128, 128), dtype=np.float32)  # [r, j] = h*W1[j, r%16]
    for r in range(128):
        w1selt[r, :] = HSTEP * W1[:, r % 16]
    cst["W1SelT"] = w1selt
    cst["W1hT"] = (HSTEP * W1.T)                                   # (16,128)
    # output reconstruction: z = pinv(W1) @ hpre  (W1 is 128x16, cond ~2)
    R = np.linalg.pinv(W1.astype(np.float64)).astype(np.float32)   # (16,128)
    cst["RT"] = np.ascontiguousarray(R.T)                          # (128,16)
    return {k: v.astype(np.float32) for k, v in cst.items()}


def _build(n_intervals=NI):
    """Build + compile the Bass module (cached per interval count)."""
    key = n_intervals
    if key in _BUILD_CACHE:
        return _BUILD_CACHE[key]

    import concourse.bass as bass
    import concourse.bacc as bacc
    import concourse.tile as tile
    from concourse import mybir

    F32 = mybir.dt.float32
    TANH = mybir.ActivationFunctionType.Tanh
    MULT = mybir.AluOpType.mult
    ADD = mybir.AluOpType.add

    nc = bacc.Bacc("TRN2", target_bir_lowering=False, debug=False,
                   num_devices=N_CORES)

    d_us3 = nc.dram_tensor("us3", (n_intervals, 24, B_LOC), F32, kind="ExternalInput")
    d_ys0 = nc.dram_tensor("ys0T", (16, B_LOC), F32, kind="ExternalInput")
    d_W1T = nc.dram_tensor("W1T", (16, 128), F32, kind="ExternalInput")
    d_W2aT = nc.dram_tensor("W2aT", (128, 128), F32, kind="ExternalInput")
    d_W2bT = nc.dram_tensor("W2bT", (128, 16), F32, kind="ExternalInput")
    d_b1 = nc.dram_tensor("b1c", (128, 1), F32, kind="ExternalInput")
    d_b2c = nc.dram_tensor("b2c", (128, 1), F32, kind="ExternalInput")
    d_b2t = nc.dram_tensor("b2t", (16, 1), F32, kind="ExternalInput")
    d_Abc = nc.dram_tensor("Abc", (4, 24, 128), F32, kind="ExternalInput")
    d_W1SelT = nc.dram_tensor("W1SelT", (128, 128), F32, kind="ExternalInput")
    d_W1hT = nc.dram_tensor("W1hT", (16, 128), F32, kind="ExternalInput")
    d_RT = nc.dram_tensor("RT", (128, 16), F32, kind="ExternalInput")
    d_out = nc.dram_tensor("out", (n_intervals, 16, B_LOC), F32, kind="ExternalOutput")

    with tile.TileContext(nc) as tc:
        with (
            tc.tile_pool(name="consts", bufs=1) as consts,
            tc.tile_pool(name="zpool", bufs=3) as zpool,
            tc.tile_pool(name="work", bufs=2) as work,
            tc.tile_pool(name="u3p", bufs=3) as u3p,
            tc.tile_pool(name="ps1", bufs=1, space="PSUM") as ps1,
            tc.tile_pool(name="ps2", bufs=2, space="PSUM") as ps2,
        ):
            W1T = consts.tile([16, 128], F32)
            W2aT = consts.tile([128, 128], F32)
            W2bT = consts.tile([128, 16], F32)
            b1c = consts.tile([128, 1], F32)
            b2c = consts.tile([128, 1], F32)
            b2t = consts.tile([16, 1], F32)
            W1SelT = consts.tile([128, 128], F32)
            W1hT = consts.tile([16, 128], F32)
            RT = consts.tile([128, 16], F32)
            Abc = [consts.tile([24, 128], F32, name=f"Abc{i}") for i in range(4)]
            nc.sync.dma_start(W1T[:], d_W1T.ap())
            nc.sync.dma_start(W2aT[:], d_W2aT.ap())
            nc.sync.dma_start(W2bT[:], d_W2bT.ap())
            nc.sync.dma_start(b1c[:], d_b1.ap())
            nc.sync.dma_start(b2c[:], d_b2c.ap())
            nc.sync.dma_start(b2t[:], d_b2t.ap())
            nc.sync.dma_start(W1SelT[:], d_W1SelT.ap())
            nc.sync.dma_start(W1hT[:], d_W1hT.ap())
            nc.sync.dma_start(RT[:], d_RT.ap())
            for i in range(4):
                nc.sync.dma_start(Abc[i][:], d_Abc.ap()[i])

            z0 = zpool.tile([16, B_LOC], F32, tag="z")
            nc.sync.dma_start(z0[:], d_ys0.ap())

            # hpre is THE state: a persistent PSUM accumulator holding W1 @ z.
            # Each substep adds W1 @ dz via one K=128 + one K=16 matmul; z is
            # only reconstructed per interval for output via R = pinv(W1).
            hpre = ps1.tile([128, B_LOC], F32, tag="hpre")
            nc.tensor.matmul(hpre[:], W1T[:], z0[:], start=True, stop=False,
                             skip_group_check=True)

            HB = B_LOC // 2
            u3s = {}

            def load_u3(k):
                if k < n_intervals:
                    t = u3p.tile([24, B_LOC], F32, tag="u3", name=f"u3_{k}")
                    nc.sync.dma_start(t[:], d_us3.ap()[k])
                    u3s[k] = t

            load_u3(0)
            load_u3(1)
            for k in range(n_intervals):
                load_u3(k + 2)
                u3 = u3s.pop(k)
                for i in range(4):
                    dXb = ps2.tile([128, B_LOC], F32, tag="dXb")
                    # dXb halves fill the PE's tanh windows
                    nc.tensor.matmul(dXb[:, :HB], Abc[i][:], u3[:, :HB],
                                     start=True, stop=True)
                    th = work.tile([128, B_LOC], F32, tag="th")
                    # column-split pipeline: tanh_h half 0 -> MM2a half 0
                    # overlaps tanh_h half 1 -> MM2a half 1
                    nc.scalar.activation(th[:, :HB], hpre[:, :HB], TANH, bias=b1c[:])
                    nc.scalar.activation(th[:, HB:], hpre[:, HB:], TANH, bias=b1c[:])
                    vfc_h = [ps1.tile([128, HB], F32, tag=f"vfc{h}", name=f"vfc{h}_{k}_{i}")
                             for h in range(2)]
                    nc.tensor.matmul(vfc_h[0][:], W2aT[:], th[:, :HB],
                                     start=True, stop=True)
                    nc.tensor.matmul(vfc_h[1][:], W2aT[:], th[:, HB:],
                                     start=True, stop=True)
                    vft_ps = ps1.tile([16, B_LOC], F32, tag="vft")
                    nc.tensor.matmul(vft_ps[:], W2bT[:], th[:], start=True, stop=True)
                    nc.tensor.matmul(dXb[:, HB:], Abc[i][:], u3[:, HB:],
                                     start=True, stop=True)
                    vfc = work.tile([128, B_LOC], F32, tag="vfcs")
                    nc.scalar.activation(vfc[:, :HB], vfc_h[0][:], TANH, bias=b2c[:])
                    nc.scalar.activation(vfc[:, HB:], vfc_h[1][:], TANH, bias=b2c[:])
                    vft = work.tile([16, B_LOC], F32, tag="vfts")
                    nc.scalar.activation(vft[:], vft_ps[:], TANH, bias=b2t[:])
                    tmp = work.tile([128, B_LOC], F32, tag="tmp")
                    nc.vector.tensor_tensor(tmp[:, :HB], vfc[:, :HB], dXb[:, :HB],
                                            MULT)
                    # chain-critical state update, half-pipelined:
                    # hpre += (h*W1*Sel^T)@tmp + (h*W1)@vft
                    nc.tensor.matmul(hpre[:, :HB], W1SelT[:], tmp[:, :HB],
                                     start=False, stop=False, skip_group_check=True)
                    nc.vector.tensor_tensor(tmp[:, HB:], vfc[:, HB:], dXb[:, HB:],
                                            MULT)
                    nc.tensor.matmul(hpre[:, HB:], W1SelT[:], tmp[:, HB:],
                                     start=False, stop=False, skip_group_check=True)
                    nc.tensor.matmul(hpre[:], W1hT[:], vft[:], start=False,
                                     stop=False, skip_group_check=True)
                # per-interval output: z_{k+1} = pinv(W1) @ hpre
                hps = work.tile([128, B_LOC], F32, tag="hps")
                nc.vector.tensor_copy(hps[:], hpre[:])
                zt_ps = ps2.tile([16, B_LOC], F32, tag="ztp")
                nc.tensor.matmul(zt_ps[:], RT[:], hps[:], start=True, stop=True)
                zout = zpool.tile([16, B_LOC], F32, tag="z")
                nc.vector.tensor_copy(zout[:], zt_ps[:])
                nc.sync.dma_start(d_out.ap()[k], zout[:])

    nc.compile()
    _BUILD_CACHE[key] = nc
    return nc


def _prep_core_inputs(us, ys, cst, core, n_intervals):
    b0 = core * B_LOC
    usc = np.ascontiguousarray(us[:, b0:b0 + B_LOC, :].transpose(0, 2, 1))  # (L,8,B)
    us_ext = np.concatenate([2.0 * usc[:1] - usc[1:2], usc], axis=0)  # (L+1,8,B)
    sw = np.lib.stride_tricks.sliding_window_view(us_ext, 3, axis=0)  # (L-1,8,B,3)
    us3 = np.ascontiguousarray(sw.transpose(0, 3, 1, 2).reshape(L - 1, 24, B_LOC))
    us3 = us3[:n_intervals].astype(np.float32)
    ys0T = np.ascontiguousarray(ys[0, b0:b0 + B_LOC, :].T).astype(np.float32)
    m = {"us3": us3, "ys0T": ys0T}
    m.update({k: v for k, v in cst.items() if k not in ("Abc",)})
    m["Abc"] = cst["Abc"]
    return m


def kernel(ts, us, ys, W1, b1, W2, b2, batch_size=None, n_intervals=NI):
    from concourse.bass_utils import run_bass_kernel_spmd

    us = np.asarray(us, dtype=np.float32)
    ys = np.asarray(ys, dtype=np.float32)
    cst = _host_constants(np.asarray(W1, np.float32), np.asarray(b1, np.float32),
                          np.asarray(W2, np.float32), np.asarray(b2, np.float32))
    nc = _build(n_intervals)
    in_maps = [_prep_core_inputs(us, ys, cst, c, n_intervals) for c in range(N_CORES)]
    res = run_bass_kernel_spmd(nc, in_maps, core_ids=list(range(N_CORES)))
    out = np.empty((B_TOT, n_intervals + 1, Y), dtype=np.float32)
    out[:, 0, :] = ys[0]
    for c in range(N_CORES):
        b0 = c * B_LOC
        out[b0:b0 + B_LOC, 1:, :] = res.results[c]["out"].transpose(2, 0, 1)
    kernel._last_results = res
    return out
